# revision 1
# baseline (speedup 1.0000x reference)
"""Multi-head attention + residual + LayerNorm on 8 Trainium2 NeuronCores.

Reference computation (B=2, S=2048, D=1024, H=16, HD=64):
    q,k,v = split_heads(x@Wq+bq), ...       # [B,H,S,HD]
    attn  = softmax(q k^T / sqrt(HD))
    out   = (attn v) merged -> [B,S,D] @ Wp + bp
    y     = LayerNorm(x + out) * gamma + beta

Sharding: 8 cores = 2 batches x 4 query-slices of 512 rows.
Each core computes QKV projections for its 512-row slice; K^T and V
slices are AllGathered across the 4 cores of the same batch in 8
combined per-head-pair pieces (so attention pipelines behind the
gathers), then each core runs attention for all 16 heads restricted to
its 512 queries and finishes with projection + residual + LayerNorm.

Numerics: attention path in bf16 (matmuls accumulate f32 in PSUM),
residual + LayerNorm in f32.  Validated vs the f32 reference:
max abs err ~2.5e-3 on output scale ~5 (rel ~5e-4).
"""

import os

import ml_dtypes
import numpy as np

import concourse.bacc as bacc
import concourse.tile as tile
from concourse import mybir
from concourse.bass_utils import run_bass_kernel_spmd

B, S, D, H, HD = 2, 2048, 1024, 16, 64
EPS = 1e-5
NCORES = 8
SL = S // 4          # 512 query rows per core
GROUPS = [[0, 1, 2, 3], [4, 5, 6, 7]]
BF = mybir.dt.bfloat16
F32 = mybir.dt.float32
Act = mybir.ActivationFunctionType
Alu = mybir.AluOpType


def build_program():
    nc = bacc.Bacc("TRN2", target_bir_lowering=False, debug=False,
                   num_devices=NCORES)

    # ---- I/O ----
    xT_d = nc.dram_tensor("xT", [D, SL], BF, kind="ExternalInput")
    xq_d = nc.dram_tensor("xq", [SL, D], F32, kind="ExternalInput")
    wq_d = nc.dram_tensor("wq", [D, D], BF, kind="ExternalInput")
    wk_d = nc.dram_tensor("wk", [D, D], BF, kind="ExternalInput")
    wv_d = nc.dram_tensor("wv", [D, D], BF, kind="ExternalInput")
    wp_d = nc.dram_tensor("wp", [D, D], BF, kind="ExternalInput")
    bq_d = nc.dram_tensor("bq", [D], F32, kind="ExternalInput")
    bk_d = nc.dram_tensor("bk", [D], F32, kind="ExternalInput")
    bv_d = nc.dram_tensor("bv", [D], F32, kind="ExternalInput")
    bp_d = nc.dram_tensor("bp", [D], BF, kind="ExternalInput")
    gamma_d = nc.dram_tensor("gamma", [D], F32, kind="ExternalInput")
    beta_d = nc.dram_tensor("beta", [D], F32, kind="ExternalInput")
    y_d = nc.dram_tensor("y", [SL, D], F32, kind="ExternalOutput")

    import concourse.bass as bass

    def bcast_ap(dram_t, parts=128):
        # replicate a [D] dram vector across `parts` partitions
        return bass.AP(tensor=dram_t, offset=0, ap=[[0, parts], [1, D]])

    with tile.TileContext(nc) as tc:
        with (
            tc.tile_pool(name="persist", bufs=1) as persist,
            tc.tile_pool(name="dram", bufs=1, space="DRAM") as dram,
        ):
            # persistent tiles
            qt_sb = persist.tile([128, 8, SL], BF)        # Q^T
            xq_sb = persist.tile([128, 4, D], F32)        # natural x slice
            wp_sb = persist.tile([128, 8, 2, 512], BF)
            outT_sb = persist.tile([128, 8, SL], BF)      # normalized attn out^T
            bv_bc = persist.tile([128, D], F32)
            gamma_bc = persist.tile([128, D], F32)
            beta_bc = persist.tile([128, D], F32)
            bq_sb = persist.tile([128, 8], F32)
            bk_sb = persist.tile([128, 8], F32)
            bp_sb = persist.tile([1, D], BF)
            ones_sb = persist.tile([1, 128], BF)
            eps_sb = persist.tile([128, 1], F32)
            # manually double-buffered V tiles so the ones columns are
            # memset once: layout [V_A | 1 | V_B | 1] per k-chunk
            vh_tiles = [persist.tile([128, 16, 130], BF, name=f"vh{i}") for i in range(2)]

            # DRAM scratch: combined K+V bounce/gather per head pair.
            # block 0 = K^T rows [hp*128,(hp+1)*128) flat [p, s] (p-major);
            # block 1 = V cols  [hp*128,(hp+1)*128) flat [s, c] (s-major).
            kvb_hp = [dram.tile([2, 128 * SL], BF, name=f"kvb{i}") for i in range(8)]
            kvg_hp = [dram.tile([4, 2, 128 * SL], BF, name=f"kvg{i}") for i in range(8)]
            dumb_in = dram.tile([1, 128], BF, name="dumb_in")
            dumb_out = dram.tile([4, 128], BF, name="dumb_out")
            nc.gpsimd.collective_compute(
                "AllGather", Alu.bypass, replica_groups=GROUPS,
                ins=[dumb_in[:].opt()], outs=[dumb_out[:].opt()])

            # small/early loads first (biases feed phase-1 epilogues)
            nc.sync.dma_start(bk_sb[:], bk_d.ap().rearrange("(co p) -> p co", p=128))
            nc.sync.dma_start(bv_bc[:], bcast_ap(bv_d))
            nc.sync.dma_start(bq_sb[:], bq_d.ap().rearrange("(co p) -> p co", p=128))
            nc.vector.memset(ones_sb[:], 1.0)
            nc.vector.memset(eps_sb[:], EPS)
            for t in vh_tiles:
                nc.vector.memset(t[:, :, 64:65], 1.0)
                nc.vector.memset(t[:, :, 129:130], 1.0)

            # ---------------- phase 1: QKV projections for this slice ----------------
            with (
                tc.tile_pool(name="ph1w", bufs=1) as ph1w,
                tc.tile_pool(name="ph1", bufs=3) as ph1,
                tc.tile_pool(name="psum1", bufs=4, space="PSUM") as psum1,
            ):
                # per-chunk loads, strictly in need-order on the sync queue
                xt_c, wk_c, wv_c, wq_c = [], [], [], []
                for ci in range(8):
                    xt = ph1w.tile([128, SL], BF, tag=f"xt{ci}")
                    nc.sync.dma_start(xt[:], xT_d[ci * 128:(ci + 1) * 128, :])
                    xt_c.append(xt)
                    wk = ph1w.tile([128, 8, 128], BF, tag=f"wk{ci}")
                    nc.sync.dma_start(wk[:], wk_d[ci * 128:(ci + 1) * 128, :]
                                      .rearrange("p (co q) -> p co q", q=128))
                    wk_c.append(wk)
                for ci in range(8):
                    wv = ph1w.tile([128, 2, 512], BF, tag=f"wv{ci}")
                    nc.sync.dma_start(wv[:], wv_d[ci * 128:(ci + 1) * 128, :]
                                      .rearrange("p (dh q) -> p dh q", q=512))
                    wv_c.append(wv)
                for ci in range(8):
                    wq = ph1w.tile([128, 8, 128], BF, tag=f"wq{ci}")
                    nc.sync.dma_start(wq[:], wq_d[ci * 128:(ci + 1) * 128, :]
                                      .rearrange("p (co q) -> p co q", q=128))
                    wq_c.append(wq)
                # loads needed later still on sync, after the critical ones
                nc.sync.dma_start(xq_sb[:], xq_d.ap().rearrange("(qc p) d -> p qc d", p=128))
                nc.sync.dma_start(wp_sb[:], wp_d.ap().rearrange("(hp p) (dh q) -> p hp dh q", p=128, q=512))
                nc.sync.dma_start(gamma_bc[:], bcast_ap(gamma_d))
                nc.sync.dma_start(beta_bc[:], bcast_ap(beta_d))
                nc.sync.dma_start(bp_sb[:], bp_d.ap().rearrange("(o d) -> o d", o=1))

                def k_chunk(co):
                    ps = psum1.tile([128, SL], F32, tag="ps1", name=f"psk{co}")
                    for ci in range(8):
                        nc.tensor.matmul(ps[:], wk_c[ci][:, co, :], xt_c[ci][:],
                                         start=(ci == 0), stop=(ci == 7))
                    kt_t = ph1.tile([128, SL], BF, tag="kt", name=f"ktt{co}")
                    nc.vector.tensor_scalar_add(kt_t[:], ps[:], bk_sb[:, co:co + 1])
                    nc.gpsimd.dma_start(
                        kvb_hp[co][0, :].rearrange("(p s) -> p s", p=128), kt_t[:])

                def v_half(dh):
                    for sc in range(4):
                        ps = psum1.tile([128, 512], F32, tag="ps1", name=f"psv{dh}{sc}")
                        for ci in range(8):
                            nc.tensor.matmul(ps[:], xt_c[ci][:, sc * 128:(sc + 1) * 128],
                                             wv_c[ci][:, dh, :],
                                             start=(ci == 0), stop=(ci == 7))
                        v_t = ph1.tile([128, 512], BF, tag="vt", name=f"vtt{dh}{sc}")
                        nc.vector.tensor_add(v_t[:], ps[:], bv_bc[:, dh * 512:(dh + 1) * 512])
                        for g4 in range(4):
                            hp = 4 * dh + g4
                            dst = kvb_hp[hp][1, :].rearrange(
                                "(sc p c) -> sc p c", sc=4, p=128)
                            nc.gpsimd.dma_start(dst[sc], v_t[:, g4 * 128:(g4 + 1) * 128])

                def fire_cc(hp):
                    nc.gpsimd.collective_compute(
                        "AllGather", Alu.bypass, replica_groups=GROUPS,
                        ins=[kvb_hp[hp][:].opt()], outs=[kvg_hp[hp][:].opt()])

                for co in range(4):
                    k_chunk(co)
                v_half(0)
                for hp in range(4):
                    fire_cc(hp)
                for co in range(4, 8):
                    k_chunk(co)
                v_half(1)
                for hp in range(4, 8):
                    fire_cc(hp)

                # Q^T (local only; overlaps the gathers)
                for co in range(8):
                    ps = psum1.tile([128, SL], F32, tag="ps1", name=f"psq{co}")
                    for ci in range(8):
                        nc.tensor.matmul(ps[:], wq_c[ci][:, co, :], xt_c[ci][:],
                                         start=(ci == 0), stop=(ci == 7))
                    nc.vector.tensor_scalar_add(qt_sb[:, co, :], ps[:], bq_sb[:, co:co + 1])

            # ---------------- phase 2: attention, one head pair at a time ----------------
            with (
                tc.tile_pool(name="kv", bufs=2) as kv,
                tc.tile_pool(name="expp", bufs=2) as expp,
                tc.tile_pool(name="small", bufs=3) as small,
                tc.tile_pool(name="ps_sc", bufs=2, space="PSUM") as ps_sc,
                tc.tile_pool(name="ps_o", bufs=1, space="PSUM") as ps_o,
                tc.tile_pool(name="ps_rb", bufs=1, space="PSUM") as ps_rb,
            ):
                for hp in range(8):
                    kth_t = kv.tile([128, 16, 128], BF, tag="kth")
                    vh_t = vh_tiles[hp % 2]
                    for j in range(4):
                        ksrc = kvg_hp[hp][j, 0, :].rearrange(
                            "(p k4 m) -> p k4 m", p=128, m=128)
                        nc.sync.dma_start(kth_t[:, 4 * j:4 * (j + 1), :], ksrc)
                        vsrc = kvg_hp[hp][j, 1, :].rearrange(
                            "(k4 p c) -> p k4 c", k4=4, p=128)
                        nc.sync.dma_start(vh_t[:, 4 * j:4 * (j + 1), 0:64], vsrc[:, :, 0:64])
                        nc.sync.dma_start(vh_t[:, 4 * j:4 * (j + 1), 65:129], vsrc[:, :, 64:128])

                    exp_t = expp.tile([128, 16, 2, 512], BF, tag="exp")
                    oAB = ps_o.tile([65, 2, SL], F32, tag="oAB")
                    # scores + exp for the whole pair first: keeps the PE queue
                    # free of V-gather-dependent work until V actually arrives
                    for kc in range(16):
                        ps = ps_sc.tile([128, 2, 512], F32, tag="sc")
                        # head A on PE rows 0-63, head B on rows 64-127 (concurrent)
                        nc.tensor.matmul(ps[:, 0, :], kth_t[0:64, kc, :],
                                         qt_sb[0:64, hp, :], start=True, stop=True)
                        nc.tensor.matmul(ps[:, 1, :], kth_t[64:128, kc, :],
                                         qt_sb[64:128, hp, :], start=True, stop=True)
                        nc.scalar.activation(exp_t[:, kc, :, :], ps[:], Act.Exp, scale=0.125)
                    for kc in range(16):
                        nc.tensor.matmul(oAB[:, 0, :], vh_t[:, kc, 0:65],
                                         exp_t[:, kc, 0, :],
                                         start=(kc == 0), stop=(kc == 15))
                        nc.tensor.matmul(oAB[:, 1, :], vh_t[:, kc, 65:130],
                                         exp_t[:, kc, 1, :],
                                         start=(kc == 0), stop=(kc == 15))

                    # softmax normalization: rows 0-63 = head dims, row 64 = sum(exp)
                    sums_sb = small.tile([128, 2, SL], F32, tag="sums")
                    nc.vector.tensor_copy(sums_sb[64:65, :, :], oAB[64:65, :, :])
                    sAB = small.tile([2, SL], F32, tag="sAB")
                    nc.gpsimd.dma_start(sAB[0:1, :], sums_sb[64:65, 0, :])
                    nc.gpsimd.dma_start(sAB[1:2, :], sums_sb[64:65, 1, :])
                    rABf = small.tile([2, SL], F32, tag="rABf")
                    nc.vector.reciprocal_approx_fast(out=rABf[:], in_=sAB[:])
                    rAB = small.tile([2, SL], BF, tag="rAB")
                    with nc.allow_low_precision("softmax scale in bf16"):
                        nc.vector.tensor_copy(rAB[:], rABf[:])
                    rB0 = small.tile([1, SL], BF, tag="rB0")
                    nc.gpsimd.dma_start(rB0[:], rAB[1:2, :])
                    rbA = ps_rb.tile([64, SL], F32, tag="rbA")
                    rbB = ps_rb.tile([64, SL], F32, tag="rbB")
                    nc.tensor.matmul(rbA[:], ones_sb[0:1, 0:64], rAB[0:1, :], start=True, stop=True)
                    nc.tensor.matmul(rbB[:], ones_sb[0:1, 0:64], rB0[:], start=True, stop=True)
                    # DVE can read only one PSUM operand per op: stage oAB in SBUF
                    oSB = small.tile([64, 2, SL], F32, tag="oSB")
                    nc.vector.tensor_copy(oSB[:], oAB[0:64, :, :])
                    nc.vector.tensor_mul(outT_sb[0:64, hp, :], oSB[:, 0, :], rbA[:])
                    tmpB = small.tile([64, SL], BF, tag="tmpB")
                    nc.vector.tensor_mul(tmpB[:], oSB[:, 1, :], rbB[:])
                    nc.gpsimd.dma_start(outT_sb[64:128, hp, :], tmpB[:])

            # ---------------- phase 3: out-projection + residual + LayerNorm ----------------
            with (
                tc.tile_pool(name="ph3", bufs=3) as ph3,
                tc.tile_pool(name="ph3s", bufs=4) as ph3s,
                tc.tile_pool(name="psum3", bufs=4, space="PSUM") as psum3,
            ):
                for qc in range(4):
                    y_t = ph3.tile([128, D], F32, tag="y")
                    for dh in range(2):
                        ps = psum3.tile([128, 512], F32, tag="py")
                        for hp in range(8):
                            nc.tensor.matmul(ps[:], outT_sb[:, hp, qc * 128:(qc + 1) * 128],
                                             wp_sb[:, hp, dh, :],
                                             start=(hp == 0), stop=False)
                        # + bp via a rank-1 matmul with a ones row
                        nc.tensor.matmul(ps[:], ones_sb[0:1, :],
                                         bp_sb[0:1, dh * 512:(dh + 1) * 512],
                                         start=False, stop=True)
                        nc.vector.tensor_add(y_t[:, dh * 512:(dh + 1) * 512], ps[:],
                                             xq_sb[:, qc, dh * 512:(dh + 1) * 512])
                    # LayerNorm over D=1024
                    stats = ph3s.tile([128, 2, 6], F32, tag="stats")
                    nc.vector.bn_stats(stats[:, 0, :], y_t[:, 0:512])
                    nc.vector.bn_stats(stats[:, 1, :], y_t[:, 512:1024])
                    mv = ph3s.tile([128, 2], F32, tag="mv")
                    nc.vector.bn_aggr(mv[:], stats[:])
                    rstd = ph3s.tile([128, 1], F32, tag="rstd")
                    nc.scalar.activation(rstd[:], mv[:, 1:2], Act.Sqrt, bias=eps_sb[:])
                    nc.vector.reciprocal(rstd[:], rstd[:])
                    # y = ((y - mu) * gamma) * rstd + beta
                    nc.vector.scalar_tensor_tensor(
                        out=y_t[:], in0=y_t[:], scalar=mv[:, 0:1], in1=gamma_bc[:],
                        op0=Alu.subtract, op1=Alu.mult)
                    nc.vector.scalar_tensor_tensor(
                        out=y_t[:], in0=y_t[:], scalar=rstd[:], in1=beta_bc[:],
                        op0=Alu.mult, op1=Alu.add)
                    nc.sync.dma_start(y_d[qc * 128:(qc + 1) * 128, :], y_t[:])

    nc.compile()
    return nc


_PROGRAM = None


def _get_program():
    global _PROGRAM
    if _PROGRAM is None:
        _PROGRAM = build_program()
    return _PROGRAM


def kernel(**inputs):
    x = np.asarray(inputs["x"], np.float32)
    bf = ml_dtypes.bfloat16
    shared = {
        "wq": np.asarray(inputs["Wq"], np.float32).astype(bf),
        "wk": np.asarray(inputs["Wk"], np.float32).astype(bf),
        "wv": np.asarray(inputs["Wv"], np.float32).astype(bf),
        "wp": np.asarray(inputs["Wp"], np.float32).astype(bf),
        "bq": np.asarray(inputs["bq"], np.float32),
        "bk": np.asarray(inputs["bk"], np.float32),
        "bv": np.asarray(inputs["bv"], np.float32),
        "bp": np.asarray(inputs["bp"], np.float32).astype(bf),
        "gamma": np.asarray(inputs["gamma"], np.float32),
        "beta": np.asarray(inputs["beta"], np.float32),
    }
    in_maps = []
    for c in range(NCORES):
        b, i = c // 4, c % 4
        xs = np.ascontiguousarray(x[b, i * SL:(i + 1) * SL, :])
        m = dict(shared)
        m["xT"] = np.ascontiguousarray(xs.T).astype(bf)
        m["xq"] = xs
        in_maps.append(m)

    nc = _get_program()
    trace_dir = os.environ.get("BASS_KERNEL_TRACE_DIR")
    kwargs = {}
    if trace_dir:
        kwargs = {"trace": True, "tmpdir": trace_dir}
    res = run_bass_kernel_spmd(nc, in_maps, core_ids=list(range(NCORES)), **kwargs)

    out = np.empty((B, S, D), np.float32)
    for c in range(NCORES):
        b, i = c // 4, c % 4
        out[b, i * SL:(i + 1) * SL, :] = res.results[c]["y"]
    if trace_dir:
        kernel.last_exec_time_ns = res.exec_time_ns
        kernel.last_trace = res.instructions_and_trace
    return out



# revision 14
# speedup vs baseline: 1.1845x; 1.1845x over previous
"""Multi-head attention + residual + LayerNorm on 8 Trainium2 NeuronCores.

Reference computation (B=2, S=2048, D=1024, H=16, HD=64):
    q,k,v = split_heads(x@Wq+bq), ...       # [B,H,S,HD]
    attn  = softmax(q k^T / sqrt(HD))
    out   = (attn v) merged -> [B,S,D] @ Wp + bp
    y     = LayerNorm(x + out) * gamma + beta

Sharding: 8 cores = 2 batches x 4 query-slices of 512 rows.  No
collectives: each core recomputes K,V for its full batch (cheap in fp8
DoubleRow) and runs attention for its 512 queries across all 16 heads.

Numerics: all matmuls in fp8(e4m3) with DoubleRow perf mode (2 k-tiles
per pass, 0.5 cyc/col).  exp() emitted as fp8 directly; part of the exp
work runs on DVE via a Schraudolph bit-trick (uint8 saturating
mul+add, bitcast to fp8).  Residual + LayerNorm in f32.
"""

import os

import ml_dtypes
import numpy as np

import concourse.bacc as bacc
import concourse.tile as tile
from concourse import mybir
from concourse.bass_utils import run_bass_kernel_spmd

B, S, D, H, HD = 2, 2048, 1024, 16, 64
EPS = 1e-5
NCORES = 8
SL = S // 4              # 512 query rows per core
F32 = mybir.dt.float32
BF = mybir.dt.bfloat16
FP8 = mybir.dt.float8e4
U8 = mybir.dt.uint8
Act = mybir.ActivationFunctionType
Alu = mybir.AluOpType
PM = mybir.MatmulPerfMode
E4 = ml_dtypes.float8_e4m3fn

# exp(s/8 - ln16): keeps fp8 range safe; numerator and denominator of the
# softmax scale together so the ratio is unchanged.
EXP_SCALE = 0.125
EXP_BIAS = -2.772588722239781  # -ln(16)
# Schraudolph constants for fp8(e4m3) bits = 8*(log2(v) + 7),
# v = exp(s/8 - ln16):  bits = s*log2(e) + 24.
SCH_MUL = float(np.log2(np.e))
SCH_ADD = 24.0
# score kc-slices whose exp runs on DVE (Schraudolph) instead of ACT
DVE_KCS = (1, 4, 7, 10, 13)


def build_program():
    nc = bacc.Bacc("TRN2", target_bir_lowering=False, debug=False,
                   num_devices=NCORES)

    # ---- I/O ----
    xT_d = nc.dram_tensor("xT", [128, 4, 2, S], FP8, kind="ExternalInput")
    xq_d = nc.dram_tensor("xq", [128, 4, D], BF, kind="ExternalInput")
    wq_d = nc.dram_tensor("wq", [128, 4, 2, D], FP8, kind="ExternalInput")
    wk_d = nc.dram_tensor("wk", [128, 4, 2, D], FP8, kind="ExternalInput")
    wv_d = nc.dram_tensor("wv", [128, 4, 2, D], FP8, kind="ExternalInput")
    wp_d = nc.dram_tensor("wp", [128, 4, 2, D], FP8, kind="ExternalInput")
    bq_d = nc.dram_tensor("bq", [128, 8], F32, kind="ExternalInput")
    bk_d = nc.dram_tensor("bk", [128, 8], F32, kind="ExternalInput")
    bv_d = nc.dram_tensor("bv", [1, D], BF, kind="ExternalInput")
    bp_d = nc.dram_tensor("bp", [1, D], BF, kind="ExternalInput")
    gamma_d = nc.dram_tensor("gamma", [D], BF, kind="ExternalInput")
    beta_d = nc.dram_tensor("beta", [D], BF, kind="ExternalInput")
    y_d = nc.dram_tensor("y", [SL, D], F32, kind="ExternalOutput")

    import concourse.bass as bass

    def bcast_ap(dram_t, parts=128):
        return bass.AP(tensor=dram_t, offset=0, ap=[[0, parts], [1, D]])

    with tile.TileContext(nc) as tc:
        with (
            tc.tile_pool(name="persist", bufs=1) as persist,
        ):
            # ---------------- persistent tiles ----------------
            xT_sb = persist.tile([128, 4, 2, S], FP8)      # x^T full batch
            xq_sb = persist.tile([128, 4, D], BF)          # x slice (residual)
            wq_sb = persist.tile([128, 4, 2, D], FP8)
            wk_sb = persist.tile([128, 4, 2, D], FP8)
            wv_sb = persist.tile([128, 4, 2, D], FP8)
            wp_sb = persist.tile([128, 4, 2, D], FP8)
            qstg = persist.tile([128, 8, SL], FP8)         # Q^T natural
            kt = persist.tile([128, 8, 2, S], FP8)         # K^T folded [32,2]
            qt = persist.tile([128, 8, 2, SL], FP8)        # Q^T folded
            vh = persist.tile([128, 8, 8, 2, 144], FP8)    # V + ones cols (16B-aligned tiles)
            outT = persist.tile([128, 4, 2, SL], FP8)      # normalized attn out
            bq_sb = persist.tile([128, 8], F32)
            bk_sb = persist.tile([128, 8], F32)
            bv_sb = persist.tile([1, D], BF)
            bp_sb = persist.tile([1, D], BF)
            ones_sb = persist.tile([1, 128], BF)
            gamma_bc = persist.tile([128, D], BF)
            beta_bc = persist.tile([128, D], BF)
            eps_sb = persist.tile([128, 1], F32)
            ebias_sb = persist.tile([128, 1], F32)         # exp bias -ln16
            sums_sb = persist.tile([128, 2, SL], F32)      # psum row-64 stage
            sAB = persist.tile([1, 2, SL], F32)            # softmax sums
            rAB = persist.tile([1, 2, SL], F32)            # reciprocals
            rbA = persist.tile([64, 2, SL], F32)           # broadcast recips
            tmpB = persist.tile([64, SL], FP8)

            # small setup
            nc.vector.memset(ones_sb[:], 1.0)
            nc.vector.memset(eps_sb[:], EPS)
            nc.vector.memset(ebias_sb[:], EXP_BIAS)
            for hp in range(8):
                nc.vector.memset(vh[:, hp, :, :, 64:65], 1.0)
                nc.vector.memset(vh[:, hp, :, :, 136:137], 1.0)

            # loads, in need-order
            nc.sync.dma_start(wq_sb[:], wq_d.ap())
            nc.sync.dma_start(xT_sb[:], xT_d.ap())
            nc.sync.dma_start(bq_sb[:], bq_d.ap())
            nc.sync.dma_start(bk_sb[:], bk_d.ap())
            nc.sync.dma_start(wk_sb[:], wk_d.ap())
            nc.sync.dma_start(wv_sb[:], wv_d.ap())
            nc.sync.dma_start(bv_sb[:], bv_d.ap())
            nc.sync.dma_start(xq_sb[:], xq_d.ap())
            nc.sync.dma_start(wp_sb[:], wp_d.ap())
            nc.sync.dma_start(bp_sb[:], bp_d.ap())
            nc.sync.dma_start(gamma_bc[:], bcast_ap(gamma_d))
            nc.sync.dma_start(beta_bc[:], bcast_ap(beta_d))

            with (
                tc.tile_pool(name="ps1", bufs=2, space="PSUM") as ps1,
                tc.tile_pool(name="ps_sc", bufs=2, space="PSUM") as ps_sc,
                tc.tile_pool(name="ps_o", bufs=1, space="PSUM") as ps_o,
                tc.tile_pool(name="kstgp", bufs=2) as kstgp,
                tc.tile_pool(name="expp", bufs=3) as expp,
            ):
                # ---------- phase-1 building blocks ----------
                def q_chunk(ch):
                    """Q^T dims [128ch,128ch+128) for my SL queries."""
                    ps = ps1.tile([128, 512], F32, tag="p1", name=f"q{ch}")
                    for cp in range(4):
                        nc.tensor.matmul(ps[:], wq_sb[:, cp, :, ch * 128:(ch + 1) * 128],
                                         xT_sb[:, cp, :, 0:SL],
                                         start=(cp == 0), stop=(cp == 3),
                                         perf_mode=PM.DoubleRow)
                    with nc.allow_low_precision("fp8 qt"):
                        nc.scalar.activation(qstg[:, ch, :], ps[:], Act.Identity,
                                             bias=bq_sb[:, ch:ch + 1])

                def q_remap(hp, a):
                    h = 2 * hp + a
                    nc.gpsimd.dma_start(
                        qt[64 * a:64 * a + 32, hp, :, :],
                        qstg[64 * (h % 2):64 * (h % 2) + 64, h // 2, :])

                def k_chunk(stg, ch, ks):
                    """K^T dims [128ch,...) for keys [512ks,...)."""
                    ps = ps1.tile([128, 512], F32, tag="p1", name=f"k{ch}_{ks}")
                    for cp in range(4):
                        nc.tensor.matmul(ps[:], wk_sb[:, cp, :, ch * 128:(ch + 1) * 128],
                                         xT_sb[:, cp, :, ks * 512:(ks + 1) * 512],
                                         start=(cp == 0), stop=(cp == 3),
                                         perf_mode=PM.DoubleRow)
                    with nc.allow_low_precision("fp8 kt"):
                        nc.scalar.activation(stg[:, ks * 512:(ks + 1) * 512],
                                             ps[:], Act.Identity,
                                             bias=bk_sb[:, ch:ch + 1])

                def k_remap(stg, hp, a):
                    nc.gpsimd.dma_start(
                        kt[64 * a:64 * a + 32, hp, :, :],
                        stg[64 * a:64 * a + 64, :])

                def v_chunk(kc, dh):
                    """V keys [128kc,...) dims [512dh,...) -> vh tiles."""
                    ps = ps1.tile([128, 512], F32, tag="p1", name=f"v{kc}_{dh}")
                    nc.tensor.matmul(ps[:], ones_sb[0:1, :],
                                     bv_sb[0:1, dh * 512:(dh + 1) * 512],
                                     start=True, stop=False)
                    for cp in range(4):
                        nc.tensor.matmul(ps[:], xT_sb[:, cp, :, kc * 128:(kc + 1) * 128],
                                         wv_sb[:, cp, :, dh * 512:(dh + 1) * 512],
                                         start=False, stop=(cp == 3),
                                         perf_mode=PM.DoubleRow)
                    dst = vh[:, 4 * dh:4 * dh + 4, kc // 2, kc % 2, :]
                    dst = dst.rearrange("p hp (two c) -> p hp two c", two=2)[:, :, :, 0:64]
                    with nc.allow_low_precision("fp8 vh"):
                        nc.vector.tensor_copy(
                            dst, ps[:].rearrange("p (hp two c) -> p hp two c", hp=4, two=2))

                # ---------- phase-2 building blocks ----------
                def scores_exp(expt, hp, kc):
                    ps = ps_sc.tile([128, 2, 512], F32, tag="sc", name=f"s{hp}_{kc}")
                    nc.tensor.matmul(ps[:, 0, :], kt[0:32, hp, :, kc * 128:(kc + 1) * 128],
                                     qt[0:32, hp, :, :], start=True, stop=True,
                                     perf_mode=PM.DoubleRow)
                    nc.tensor.matmul(ps[:, 1, :], kt[64:96, hp, :, kc * 128:(kc + 1) * 128],
                                     qt[64:96, hp, :, :], start=True, stop=True,
                                     perf_mode=PM.DoubleRow)
                    dst = expt[:, kc, :, :]
                    with nc.allow_low_precision("fp8 exp"):
                        if kc in DVE_KCS:
                            nc.vector.tensor_scalar(
                                out=dst.bitcast(U8), in0=ps[:],
                                scalar1=SCH_MUL, scalar2=SCH_ADD,
                                op0=Alu.mult, op1=Alu.add)
                        else:
                            nc.scalar.activation(dst, ps[:], Act.Exp,
                                                 scale=EXP_SCALE, bias=ebias_sb[:])

                def attnv(expt, hp):
                    oAB = ps_o.tile([65, 2, SL], F32, tag="o", name=f"o{hp}")
                    for j in range(8):
                        nc.tensor.matmul(oAB[:, 0, :], vh[:, hp, j, :, 0:65],
                                         expt[:, 2 * j:2 * j + 2, 0, :],
                                         start=(j == 0), stop=(j == 7),
                                         perf_mode=PM.DoubleRow)
                        nc.tensor.matmul(oAB[:, 1, :], vh[:, hp, j, :, 72:137],
                                         expt[:, 2 * j:2 * j + 2, 1, :],
                                         start=(j == 0), stop=(j == 7),
                                         perf_mode=PM.DoubleRow)
                    # normalize: row 64 holds sum(exp)
                    nc.vector.tensor_copy(sums_sb[64:65, :, :], oAB[64:65, :, :])
                    nc.gpsimd.dma_start(sAB[0:1, :, :], sums_sb[64:65, :, :])
                    nc.vector.reciprocal_approx_fast(out=rAB[:], in_=sAB[:])
                    nc.gpsimd.partition_broadcast(rbA[:, 0, :], rAB[0:1, 0, :])
                    nc.gpsimd.partition_broadcast(rbA[:, 1, :], rAB[0:1, 1, :])
                    j, t = hp // 2, hp % 2
                    with nc.allow_low_precision("fp8 outT"):
                        nc.vector.tensor_mul(outT[0:64, j, t, :], oAB[0:64, 0, :],
                                             rbA[:, 0, :])
                        nc.vector.tensor_mul(tmpB[:], oAB[0:64, 1, :], rbA[:, 1, :])
                    nc.gpsimd.dma_start(outT[64:128, j, t, :], tmpB[:])

                # ---------- interleaved schedule ----------
                for ch in range(8):
                    q_chunk(ch)
                for h in range(16):
                    q_remap(h // 2, h % 2)

                exp_tiles = {}

                def k_scores(hp):
                    stg = kstgp.tile([128, S], FP8, tag="kstg", name=f"kstg{hp}")
                    for ks in range(4):
                        k_chunk(stg, hp, ks)
                    k_remap(stg, hp, 0)
                    k_remap(stg, hp, 1)
                    expt = expp.tile([128, 16, 2, SL], FP8, tag="exp",
                                     name=f"exp{hp}")
                    exp_tiles[hp] = expt
                    for kc in range(16):
                        scores_exp(expt, hp, kc)

                k_scores(0)
                k_scores(1)
                for kc in range(16):
                    v_chunk(kc, 0)
                k_scores(2)
                attnv(exp_tiles[0], 0)
                for kc in range(16):
                    v_chunk(kc, 1)
                k_scores(3)
                attnv(exp_tiles[1], 1)
                for hp in range(4, 8):
                    k_scores(hp)
                    attnv(exp_tiles[hp - 2], hp - 2)
                attnv(exp_tiles[6], 6)
                attnv(exp_tiles[7], 7)

            # ---------------- phase 3: out-projection + residual + LN ----------------
            with (
                tc.tile_pool(name="ph3", bufs=1) as ph3,
                tc.tile_pool(name="ph3s", bufs=4) as ph3s,
                tc.tile_pool(name="psum3", bufs=4, space="PSUM") as psum3,
            ):
                for qc in range(4):
                    y_t = ph3.tile([128, D], F32, tag="y")
                    for dh in range(2):
                        ps = psum3.tile([128, 512], F32, tag="py")
                        nc.tensor.matmul(ps[:], ones_sb[0:1, :],
                                         bp_sb[0:1, dh * 512:(dh + 1) * 512],
                                         start=True, stop=False)
                        for j in range(4):
                            nc.tensor.matmul(ps[:], outT[:, j, :, qc * 128:(qc + 1) * 128],
                                             wp_sb[:, j, :, dh * 512:(dh + 1) * 512],
                                             start=False, stop=(j == 3),
                                             perf_mode=PM.DoubleRow)
                        nc.vector.tensor_add(y_t[:, dh * 512:(dh + 1) * 512], ps[:],
                                             xq_sb[:, qc, dh * 512:(dh + 1) * 512])
                    stats = ph3s.tile([128, 2, 6], F32, tag="stats")
                    nc.vector.bn_stats(stats[:, 0, :], y_t[:, 0:512])
                    nc.vector.bn_stats(stats[:, 1, :], y_t[:, 512:1024])
                    mv = ph3s.tile([128, 2], F32, tag="mv")
                    nc.vector.bn_aggr(mv[:], stats[:])
                    rstd = ph3s.tile([128, 1], F32, tag="rstd")
                    nc.scalar.activation(rstd[:], mv[:, 1:2], Act.Sqrt, bias=eps_sb[:])
                    nc.vector.reciprocal(rstd[:], rstd[:])
                    nc.vector.scalar_tensor_tensor(
                        out=y_t[:], in0=y_t[:], scalar=mv[:, 0:1], in1=gamma_bc[:],
                        op0=Alu.subtract, op1=Alu.mult)
                    nc.vector.scalar_tensor_tensor(
                        out=y_t[:], in0=y_t[:], scalar=rstd[:], in1=beta_bc[:],
                        op0=Alu.mult, op1=Alu.add)
                    nc.sync.dma_start(y_d[qc * 128:(qc + 1) * 128, :], y_t[:])

    nc.compile()
    return nc


_PROGRAM = None


def _get_program():
    global _PROGRAM
    if _PROGRAM is None:
        _PROGRAM = build_program()
    return _PROGRAM


def kernel(**inputs):
    x = np.asarray(inputs["x"], np.float32)

    def dr_fold(w):
        # [1024, N] -> [128, 4, 2, N]: row d = (2*cp + t)*128 + p
        return np.ascontiguousarray(
            w.reshape(4, 2, 128, -1).transpose(2, 0, 1, 3)).astype(E4)

    shared = {
        "wq": dr_fold(np.asarray(inputs["Wq"], np.float32)),
        "wk": dr_fold(np.asarray(inputs["Wk"], np.float32)),
        "wv": dr_fold(np.asarray(inputs["Wv"], np.float32)),
        "wp": dr_fold(np.asarray(inputs["Wp"], np.float32)),
        "bq": np.ascontiguousarray(
            np.asarray(inputs["bq"], np.float32).reshape(8, 128).T),
        "bk": np.ascontiguousarray(
            np.asarray(inputs["bk"], np.float32).reshape(8, 128).T),
        "bv": np.asarray(inputs["bv"], np.float32).reshape(1, D).astype(ml_dtypes.bfloat16),
        "bp": np.asarray(inputs["bp"], np.float32).reshape(1, D).astype(ml_dtypes.bfloat16),
        "gamma": np.asarray(inputs["gamma"], np.float32).astype(ml_dtypes.bfloat16),
        "beta": np.asarray(inputs["beta"], np.float32).astype(ml_dtypes.bfloat16),
    }
    in_maps = []
    for c in range(NCORES):
        b, i = c // 4, c % 4
        m = dict(shared)
        # x^T for this batch, query slice rotated to the front so the
        # kernel's queries are always columns [0:SL).  Keys/values are
        # rotated identically, which softmax+sum is invariant to.
        xb = x[b]
        xrot = np.concatenate([xb[i * SL:], xb[:i * SL]], axis=0)
        m["xT"] = dr_fold(xrot.T)
        m["xq"] = np.ascontiguousarray(
            xb[i * SL:(i + 1) * SL].reshape(4, 128, D).transpose(1, 0, 2)
        ).astype(ml_dtypes.bfloat16)
        in_maps.append(m)

    nc = _get_program()
    trace_dir = os.environ.get("BASS_KERNEL_TRACE_DIR")
    kwargs = {}
    if trace_dir:
        kwargs = {"trace": True, "tmpdir": trace_dir}
    res = run_bass_kernel_spmd(nc, in_maps, core_ids=list(range(NCORES)), **kwargs)

    out = np.empty((B, S, D), np.float32)
    for c in range(NCORES):
        b, i = c // 4, c % 4
        out[b, i * SL:(i + 1) * SL, :] = res.results[c]["y"]
    if trace_dir:
        kernel.last_exec_time_ns = res.exec_time_ns
        kernel.last_trace = res.instructions_and_trace
    return out
